# revision 1
# baseline (speedup 1.0000x reference)
"""LocalWindowAttention Trainium2 kernel.

Strategy: data-parallel over the 1024 (B*n_windows) windows -> 8 cores x 128
windows (2048 tokens each). Host pre-transposes x and the weights so every
matmul operand lands in SBUF with the contraction dim on partitions.

Per-core pipeline over 4 token blocks of 512:
  1. Q/K projections as float32r matmuls (stationary = weight k-tile,
     moving = x^T block), PSUM [head, 512 tokens].
  2. RoPE applied in [head_dim, token] layout via partition-swapped
     multiplies against host-built cos/sin tables (scale 1/sqrt(hd) folded
     into the Q tables); results stored bf16.
  3. V projection in bf16, stored [token, embed] (natural for AV matmul).
  4. Attention per group of 8 windows (128 tokens) x 16 heads:
     scores = Qr^T.T @ Kr^T -> PSUM [w, v]; +block-diag mask; Exp on ACT
     with fused row-sum; normalize; PE-transpose A; AV matmul -> Out^T.
  5. Output projection in bf16 from Out^T [embed, token] tiles.
"""

import json
import os
from functools import lru_cache

import numpy as np
import ml_dtypes

import concourse.bass as bass
import concourse.mybir as mybir
import concourse.tile as tile
from concourse.bass_utils import run_bass_kernel_spmd


def _split_waits_json(bir: bytes) -> bytes:
    """Walrus in this container embeds at most 1 sem-wait per instruction
    (2 for EventSemaphore). Tile freely attaches more. Spill the excess
    onto same-engine NoOps inserted right before the instruction."""
    j = json.loads(bir)
    ctr = [0]

    def cap_of(op):
        return 2 if op == "EventSemaphore" else 1

    for f in j["functions"]:
        for blk in f["blocks"]:
            out = []
            for inst in blk["instructions"]:
                si = inst.get("sync_info")
                waits = (si or {}).get("on_wait") or []
                cap = cap_of(inst.get("opcode"))
                if len(waits) > cap:
                    extra, keep = waits[:-cap], waits[-cap:]
                    for w in extra:
                        ctr[0] += 1
                        out.append({
                            "debug": inst.get("debug", 0),
                            "engine": inst["engine"],
                            "ins": [], "outs": [],
                            "name": f"I-wspill-{ctr[0]}",
                            "opcode": "NoOp",
                            "sync_info": {"on_update": [], "on_wait": [w]},
                        })
                    si["on_wait"] = keep
                out.append(inst)
            blk["instructions"] = out
    return json.dumps(j).encode()


def _patch_to_json(nc):
    orig = nc.to_json_bytes
    nc.to_json_bytes = lambda: _split_waits_json(orig())
    return nc

F32 = mybir.dt.float32
F32R = mybir.dt.float32r
BF16 = mybir.dt.bfloat16
AX = mybir.AxisListType
ALU = mybir.AluOpType
ACTF = mybir.ActivationFunctionType

B, S, D = 4, 4096, 2048
H, HD, W = 16, 128, 16
E = H * HD  # 2048
NCORES = 8
TOK_PER_CORE = B * S // NCORES  # 2048
TBLK = 512            # tokens per block
NBLK = TOK_PER_CORE // TBLK  # 4
KT = D // 128         # 16 contraction tiles
ET = E // 128         # 16 e-tiles (= heads)
NG = TBLK // 128      # 4 groups (of 8 windows) per block
MASK_NEG = -30000.0


def build_kernel(nblk=NBLK):
    nc = bass.Bass("TRN2", target_bir_lowering=False, debug=False)

    ntok = nblk * TBLK
    # DRAM I/O (per core). float32r tensors carry fp32 bits.
    xT = nc.dram_tensor("xT", [D, ntok], F32R, kind="ExternalInput")
    xTb = nc.dram_tensor("xTb", [D, ntok], BF16, kind="ExternalInput")
    wqT = nc.dram_tensor("wqT", [D, E], F32R, kind="ExternalInput")
    wkT = nc.dram_tensor("wkT", [D, E], F32R, kind="ExternalInput")
    wvTb = nc.dram_tensor("wvTb", [D, E], BF16, kind="ExternalInput")
    woTb = nc.dram_tensor("woTb", [E, D], BF16, kind="ExternalInput")
    csq = nc.dram_tensor("csq", [128, TBLK], F32, kind="ExternalInput")
    snq = nc.dram_tensor("snq", [128, TBLK], F32, kind="ExternalInput")
    csk = nc.dram_tensor("csk", [128, TBLK], F32, kind="ExternalInput")
    snk = nc.dram_tensor("snk", [128, TBLK], F32, kind="ExternalInput")
    maskd = nc.dram_tensor("maskd", [128, 4, 128], F32, kind="ExternalInput")
    idend = nc.dram_tensor("idend", [128, 128], BF16, kind="ExternalInput")
    outd = nc.dram_tensor("out", [ntok, D], F32, kind="ExternalOutput")

    with tile.TileContext(nc) as tc:
        with (
            tc.tile_pool(name="const", bufs=1) as constp,
            tc.tile_pool(name="x", bufs=1) as xpool,
            tc.tile_pool(name="xb", bufs=1) as xbpool,
            tc.tile_pool(name="wqk", bufs=3) as wqkp,
            tc.tile_pool(name="wvo", bufs=2) as wvop,
            tc.tile_pool(name="qk", bufs=1) as qkpool,
            tc.tile_pool(name="v", bufs=1) as vpool,
            tc.tile_pool(name="outT", bufs=1) as otpool,
            tc.tile_pool(name="rope", bufs=2) as ropep,
            tc.tile_pool(name="attn", bufs=2) as attnp,
            tc.tile_pool(name="small", bufs=4) as smallp,
            tc.tile_pool(name="osb", bufs=2) as osbp,
            tc.tile_pool(name="psA", bufs=2, space="PSUM") as psA,
            tc.tile_pool(name="psS", bufs=2, space="PSUM") as psS,
            tc.tile_pool(name="psT", bufs=2, space="PSUM") as psT,
            tc.tile_pool(name="psO", bufs=2, space="PSUM") as psO,
        ):
            # constants
            cs_q = constp.tile([128, TBLK], F32, tag="csq")
            sn_q = constp.tile([128, TBLK], F32, tag="snq")
            cs_k = constp.tile([128, TBLK], F32, tag="csk")
            sn_k = constp.tile([128, TBLK], F32, tag="snk")
            mask = constp.tile([128, 4, 128], F32, tag="mask")
            iden = constp.tile([128, 128], BF16, tag="iden")
            nc.sync.dma_start(cs_q[:], csq[:])
            nc.sync.dma_start(sn_q[:], snq[:])
            nc.sync.dma_start(cs_k[:], csk[:])
            nc.sync.dma_start(sn_k[:], snk[:])
            nc.sync.dma_start(mask[:], maskd[:])
            nc.sync.dma_start(iden[:], idend[:])

            for b in range(nblk):
                ts = b * TBLK
                xt = xpool.tile([128, KT, TBLK], F32R, tag="xt")
                nc.sync.dma_start(
                    xt[:],
                    xT[:, ts:ts + TBLK].rearrange("(k p) t -> p k t", p=128),
                )
                xtb = xbpool.tile([128, KT, TBLK], BF16, tag="xtb")
                nc.sync.dma_start(
                    xtb[:],
                    xTb[:, ts:ts + TBLK].rearrange("(k p) t -> p k t", p=128),
                )

                # ---- Q/K projections + RoPE -> QrT/KrT bf16 [hd, head, t]
                qrt = qkpool.tile([128, ET, TBLK], BF16, tag="qrt")
                krt = qkpool.tile([128, ET, TBLK], BF16, tag="krt")
                for wdram, cs_t, sn_t, dest in (
                    (wqT, cs_q, sn_q, qrt),
                    (wkT, cs_k, sn_k, krt),
                ):
                    for et in range(ET):
                        wt = wqkp.tile([128, KT, 128], F32R, tag="wqk")
                        nc.sync.dma_start(
                            wt[:],
                            wdram[:, et * 128:(et + 1) * 128].rearrange(
                                "(k p) e -> p k e", p=128
                            ),
                        )
                        ps = psA.tile([128, TBLK], F32, tag="proj")
                        for k in range(KT):
                            nc.tensor.matmul(
                                ps[:], wt[:, k, :], xt[:, k, :],
                                start=(k == 0), stop=(k == KT - 1),
                            )
                        # RoPE: dest = ps*cs + swap64(ps)*sn
                        rot = ropep.tile([128, TBLK], F32, tag="rot")
                        qcs = ropep.tile([128, TBLK], F32, tag="qcs")
                        nc.vector.tensor_tensor(
                            out=rot[0:64, :], in0=ps[64:128, :],
                            in1=sn_t[0:64, :], op=ALU.mult)
                        nc.vector.tensor_tensor(
                            out=rot[64:128, :], in0=ps[0:64, :],
                            in1=sn_t[64:128, :], op=ALU.mult)
                        nc.vector.tensor_tensor(
                            out=qcs[:], in0=ps[:], in1=cs_t[:], op=ALU.mult)
                        nc.vector.tensor_tensor(
                            out=dest[:, et, :], in0=qcs[:], in1=rot[:],
                            op=ALU.add)

                # ---- V projection (bf16) -> V [t, tt, e]
                vt = vpool.tile([128, NG, E], BF16, tag="vt")
                for ec in range(E // 512):
                    wv = wvop.tile([128, KT, 512], BF16, tag="wvo")
                    nc.sync.dma_start(
                        wv[:],
                        wvTb[:, ec * 512:(ec + 1) * 512].rearrange(
                            "(k p) e -> p k e", p=128
                        ),
                    )
                    for tt in range(NG):
                        ps = psA.tile([128, 512], F32, tag="proj")
                        for k in range(KT):
                            nc.tensor.matmul(
                                ps[:], xtb[:, k, tt * 128:(tt + 1) * 128],
                                wv[:, k, :],
                                start=(k == 0), stop=(k == KT - 1),
                            )
                        nc.vector.tensor_copy(
                            vt[:, tt, ec * 512:(ec + 1) * 512], ps[:])

                # ---- windowed attention, groups of 128 tokens, 4 heads/bank
                outT = otpool.tile([128, ET, TBLK], BF16, tag="outT")
                for g in range(NG):
                    gs = g * 128
                    for h0 in range(0, H, 4):
                        sps = psS.tile([128, 4, 128], F32, tag="s")
                        for i in range(4):
                            h = h0 + i
                            nc.tensor.matmul(
                                sps[:, i, :], qrt[:, h, gs:gs + 128],
                                krt[:, h, gs:gs + 128], start=True, stop=True)
                        sm = attnp.tile([128, 4, 128], F32, tag="sm")
                        nc.vector.tensor_tensor(
                            out=sm[:], in0=sps[:], in1=mask[:], op=ALU.add)
                        at = attnp.tile([128, 4, 128], BF16, tag="at")
                        for i in range(4):
                            pt = attnp.tile([128, 128], F32, tag="pt")
                            sums = smallp.tile([128, 1], F32, tag="sums")
                            nc.scalar.activation(
                                pt[:], sm[:, i, :], ACTF.Exp,
                                accum_out=sums[:])
                            rec = smallp.tile([128, 1], F32, tag="rec")
                            nc.vector.reciprocal(rec[:], sums[:])
                            nc.vector.tensor_scalar_mul(
                                at[:, i, :], pt[:], rec[:])
                        atps = psT.tile([128, 4, 128], BF16, tag="t")
                        for i in range(4):
                            nc.tensor.transpose(
                                atps[:, i, :], at[:, i, :], iden[:])
                        ats = attnp.tile([128, 4, 128], BF16, tag="ats")
                        nc.vector.tensor_copy(ats[:], atps[:])
                        ops_ = psO.tile([128, 4, 128], F32, tag="o")
                        for i in range(4):
                            h = h0 + i
                            nc.tensor.matmul(
                                ops_[:, i, :],
                                vt[:, g, h * 128:(h + 1) * 128],
                                ats[:, i, :], start=True, stop=True)
                        nc.vector.tensor_copy(
                            outT[:, h0:h0 + 4, gs:gs + 128], ops_[:])

                # ---- output projection (bf16)
                for dc in range(D // 512):
                    wo = wvop.tile([128, ET, 512], BF16, tag="wvo")
                    nc.sync.dma_start(
                        wo[:],
                        woTb[:, dc * 512:(dc + 1) * 512].rearrange(
                            "(k p) e -> p k e", p=128
                        ),
                    )
                    for tt in range(NG):
                        ps = psA.tile([128, 512], F32, tag="proj")
                        for et in range(ET):
                            nc.tensor.matmul(
                                ps[:], outT[:, et, tt * 128:(tt + 1) * 128],
                                wo[:, et, :],
                                start=(et == 0), stop=(et == ET - 1),
                            )
                        osb = osbp.tile([128, 512], F32, tag="osb")
                        nc.vector.tensor_copy(osb[:], ps[:])
                        nc.sync.dma_start(
                            outd[ts + tt * 128: ts + (tt + 1) * 128,
                                 dc * 512:(dc + 1) * 512],
                            osb[:],
                        )
    return _patch_to_json(nc)


def _host_prep(x, rope_freqs, wq, wk, wv, wo):
    bf16 = ml_dtypes.bfloat16
    x_flat = np.ascontiguousarray(x, dtype=np.float32).reshape(B * S, D)
    wqT = np.ascontiguousarray(wq.T, dtype=np.float32)
    wkT = np.ascontiguousarray(wk.T, dtype=np.float32)
    wvTb = np.ascontiguousarray(wv.T).astype(bf16)
    woTb = np.ascontiguousarray(wo.T).astype(bf16)

    f = np.asarray(rope_freqs[:W], dtype=np.float32)  # [16, 64]
    cosf, sinf = np.cos(f), np.sin(f)                 # [16, 64]
    tmod = np.arange(TBLK) % W
    cs = np.empty((128, TBLK), np.float32)
    sn = np.empty((128, TBLK), np.float32)
    p = np.arange(128)
    cs[:, :] = cosf[tmod[None, :], (p % 64)[:, None]]
    sn[:, :] = sinf[tmod[None, :], (p % 64)[:, None]]
    sn[0:64, :] *= -1.0
    scale = 1.0 / np.sqrt(np.float32(HD))
    csq, snq = cs * scale, sn * scale

    maskm = np.full((128, 128), MASK_NEG, np.float32)
    for wdw in range(128 // W):
        maskm[wdw * W:(wdw + 1) * W, wdw * W:(wdw + 1) * W] = 0.0
    maskm = np.repeat(maskm[:, None, :], 4, axis=1).copy()
    iden = np.eye(128, dtype=bf16)

    shared = dict(wqT=wqT, wkT=wkT, wvTb=wvTb, woTb=woTb,
                  csq=csq, snq=snq, csk=cs, snk=sn,
                  maskd=maskm, idend=iden)
    in_maps = []
    for c in range(NCORES):
        shard = x_flat[c * TOK_PER_CORE:(c + 1) * TOK_PER_CORE]
        xT = np.ascontiguousarray(shard.T)
        in_maps.append(dict(shared, xT=xT, xTb=xT.astype(bf16)))
    return in_maps


@lru_cache(maxsize=1)
def _get_nc():
    return build_kernel()


def kernel(x, rope_freqs, wq, wk, wv, wo):
    in_maps = _host_prep(x, rope_freqs, wq, wk, wv, wo)
    nc = _get_nc()
    res = run_bass_kernel_spmd(
        nc, in_maps, core_ids=list(range(NCORES)),
        trace=bool(int(os.environ.get("LWA_TRACE", "0"))),
    )
    if getattr(kernel, "_last_results", None) is not None or True:
        kernel._last_results = res
    out = np.concatenate([r["out"] for r in res.results], axis=0)
    return out.reshape(B, S, D)

